# revision 1
# baseline (speedup 1.0000x reference)
"""Trainium2 Bass kernel for nn_CrossAttentionBlock.

Math: with key/value seq_len == 1 the attention softmax is identically 1, so
q/k (and masked_x entirely) never affect the output:

    out[n, :] = LN(((graph_vec @ Wv.T + bv) @ Wiv.T + biv) @ Wout.T + bout)[batch_indices[n]]

i.e. a 128-row lookup table indexed by batch_indices. Strategy per core
(data-parallel over nodes, 8 cores x 50000 nodes):

  1. prologue (tiny, fp32): compute the [128, 128] table on-device
     (PE transposes + matmuls + bn_stats LayerNorm), split it into
     bf16 hi + bf16 lo parts (hi+lo ~= fp32 table to ~2^-17 rel).
  2. main loop per 512 nodes:
       - PE K=1 matmul broadcasts idx (bf16, exact for ints<256) across
         partitions into PSUM
       - DVE is_equal against a partition-iota column -> one-hot^T (bf16)
       - 4x2 PE matmuls: out[node, h] = onehotT.T @ (tbl_hi + tbl_lo)
       - ACT copies PSUM -> SBUF staging; every 8 groups one 2 MiB DMA store
"""

import sys

if "/opt/trn_rl_repo" not in sys.path:
    sys.path.insert(0, "/opt/trn_rl_repo")

import numpy as np
import ml_dtypes

import concourse.bass as bass
import concourse.bacc as bacc
import concourse.tile as tile
from concourse import mybir
from concourse import bass_utils

F32 = mybir.dt.float32
BF16 = mybir.dt.bfloat16

N_NODES = 400000
H = 128          # hidden
G = 256          # graph_dim
B = 128          # batch (table rows)
N_CORES = 8
NSHARD = N_NODES // N_CORES          # 50000
GROUP = 512                          # nodes per inner group (one PSUM bank)
NPAD = 50176                         # 98 * 512, per-core padded shard
NGROUPS = NPAD // GROUP              # 98
STORE_G = 8                          # groups per DMA store (8*512*128*4B = 2 MiB)
EPS = 1e-5

# "hilo": table as bf16 hi+lo, two accumulating matmuls (rel err ~1e-5)
# "fp16": table as single fp16, one matmul (rel err ~5e-4, less PE time)
PRECISION = "hilo"
OH_DT = BF16
# "pe": idx broadcast via K=1 PE matmul every group
# "hybrid": even store-chunks use a DMA stride-0 replicate of uint8 idx
#           (offloads ~half the broadcast work from the pacing PE engine)
BCAST = "hybrid"


def _row1(ap):
    """View a 1-D DRAM AP as [1, N]."""
    return bass.AP(tensor=ap.tensor, offset=ap.offset, ap=[[0, 1]] + list(ap.ap))


def _bcast128(ap):
    """View a 1-D DRAM AP as [128, N] replicated across partitions."""
    return bass.AP(tensor=ap.tensor, offset=ap.offset, ap=[[0, 128]] + list(ap.ap))


def build_bass(npad=NPAD, precision=None, bcast=None):
    global PRECISION, OH_DT, BCAST
    if precision is not None:
        PRECISION = precision
    if bcast is not None:
        BCAST = bcast
    OH_DT = BF16 if PRECISION == "hilo" else mybir.dt.float16
    ngroups = npad // GROUP
    nc = bacc.Bacc("TRN2", target_bir_lowering=False)

    gv_d = nc.dram_tensor("graph_vec", [B, G], F32, kind="ExternalInput")
    wv_d = nc.dram_tensor("Wv", [H, G], F32, kind="ExternalInput")
    bv_d = nc.dram_tensor("bv", [H], F32, kind="ExternalInput")
    wiv_d = nc.dram_tensor("Wiv", [H, H], F32, kind="ExternalInput")
    biv_d = nc.dram_tensor("biv", [H], F32, kind="ExternalInput")
    wout_d = nc.dram_tensor("Wout", [H, H], F32, kind="ExternalInput")
    bout_d = nc.dram_tensor("bout", [H], F32, kind="ExternalInput")
    gamma_d = nc.dram_tensor("gamma", [H], F32, kind="ExternalInput")
    beta_d = nc.dram_tensor("beta", [H], F32, kind="ExternalInput")
    eye_d = nc.dram_tensor("eye", [128, 128], F32, kind="ExternalInput")
    idx_d = nc.dram_tensor("idx", [npad], BF16, kind="ExternalInput")
    idx8_d = nc.dram_tensor("idx8", [npad], mybir.dt.uint8, kind="ExternalInput")
    out_d = nc.dram_tensor("out", [npad, H], F32, kind="ExternalOutput")

    with tile.TileContext(nc) as tc:
        with (
            tc.tile_pool(name="singles", bufs=1) as singles,
            tc.tile_pool(name="pro_ps", bufs=2, space="PSUM") as pro_ps,
            tc.tile_pool(name="oh", bufs=4) as oh_pool,
            tc.tile_pool(name="lps", bufs=3, space="PSUM") as loop_ps,
            tc.tile_pool(name="stage", bufs=2) as stage_pool,
        ):
            # ---------- constants & weights ----------
            gv_sb = singles.tile([B, G], F32, tag="gv")
            nc.sync.dma_start(out=gv_sb, in_=gv_d[:, :])
            wv_sb = singles.tile([H, G], F32, tag="wv")
            nc.sync.dma_start(out=wv_sb, in_=wv_d[:, :])
            wiv_sb = singles.tile([H, H], F32, tag="wiv")
            nc.sync.dma_start(out=wiv_sb, in_=wiv_d[:, :])
            wout_sb = singles.tile([H, H], F32, tag="wout")
            nc.sync.dma_start(out=wout_sb, in_=wout_d[:, :])
            eye_sb = singles.tile([128, 128], F32, tag="eye")
            nc.sync.dma_start(out=eye_sb, in_=eye_d[:, :])

            bv_sb = singles.tile([1, H], F32, tag="bv")
            nc.sync.dma_start(out=bv_sb, in_=_row1(bv_d[:]))
            biv_sb = singles.tile([1, H], F32, tag="biv")
            nc.sync.dma_start(out=biv_sb, in_=_row1(biv_d[:]))
            bout_sb = singles.tile([1, H], F32, tag="bout")
            nc.sync.dma_start(out=bout_sb, in_=_row1(bout_d[:]))

            gamma_gr = singles.tile([128, H], F32, tag="gamma_gr")
            nc.gpsimd.dma_start(out=gamma_gr, in_=_bcast128(gamma_d[:]))
            beta_gr = singles.tile([128, H], F32, tag="beta_gr")
            nc.gpsimd.dma_start(out=beta_gr, in_=_bcast128(beta_d[:]))

            ones32 = singles.tile([1, 128], F32, tag="ones32")
            nc.vector.memset(ones32, 1.0)
            onesbf = singles.tile([1, 128], BF16, tag="onesbf")
            nc.vector.memset(onesbf, 1.0)
            eps_sb = singles.tile([128, 1], F32, tag="eps")
            nc.vector.memset(eps_sb, EPS)

            iota_i = singles.tile([128, 1], mybir.dt.int32, tag="iota_i")
            nc.gpsimd.iota(iota_i, [[0, 1]], base=0, channel_multiplier=1)
            iota_f = singles.tile([128, 1], F32, tag="iota_f")
            nc.vector.tensor_copy(out=iota_f, in_=iota_i)

            # One barrier after all loads: PE transpose-mode matmuls have a
            # single HW wait slot, so they must not wait on >1 DMA semaphore.
            tc.strict_bb_all_engine_barrier()

            # idx loads after the barrier: its 100 KB DMA overlaps the table
            # prologue instead of delaying it.
            idx_sb = singles.tile([1, npad], BF16, tag="idx")
            nc.sync.dma_start(out=idx_sb, in_=_row1(idx_d[:]))

            # ---------- table prologue (all [128,128] fp32) ----------
            def pe_t(src, tag):
                ps = pro_ps.tile([128, 128], F32, tag="pps")
                nc.tensor.transpose(ps, src, eye_sb)
                sb = singles.tile([128, 128], F32, tag=tag)
                nc.scalar.copy(out=sb, in_=ps)
                return sb

            gv_t0 = pe_t(gv_sb[:, 0:128], "gvT0")
            gv_t1 = pe_t(gv_sb[:, 128:256], "gvT1")
            wv_t0 = pe_t(wv_sb[:, 0:128], "wvT0")
            wv_t1 = pe_t(wv_sb[:, 128:256], "wvT1")

            # v = gv @ Wv.T + bv      [b, h]
            v_ps = pro_ps.tile([128, 128], F32, tag="pps")
            nc.tensor.matmul(v_ps, gv_t0, wv_t0, start=True, stop=False)
            nc.tensor.matmul(v_ps, gv_t1, wv_t1, start=False, stop=False)
            nc.tensor.matmul(v_ps, ones32, bv_sb, start=False, stop=True)
            v_sb = singles.tile([128, 128], F32, tag="v_sb")
            nc.scalar.copy(out=v_sb, in_=v_ps)

            # v2 = v @ Wiv.T + biv    [b, j]
            v_t = pe_t(v_sb, "vT")
            wiv_t = pe_t(wiv_sb, "wivT")
            v2_ps = pro_ps.tile([128, 128], F32, tag="pps")
            nc.tensor.matmul(v2_ps, v_t, wiv_t, start=True, stop=False)
            nc.tensor.matmul(v2_ps, ones32, biv_sb, start=False, stop=True)
            v2_sb = singles.tile([128, 128], F32, tag="v2_sb")
            nc.scalar.copy(out=v2_sb, in_=v2_ps)

            # ao = v2 @ Wout.T + bout [b, h]
            v2_t = pe_t(v2_sb, "v2T")
            wout_t = pe_t(wout_sb, "woutT")
            ao_ps = pro_ps.tile([128, 128], F32, tag="pps")
            nc.tensor.matmul(ao_ps, v2_t, wout_t, start=True, stop=False)
            nc.tensor.matmul(ao_ps, ones32, bout_sb, start=False, stop=True)

            # LayerNorm over free dim
            stats = singles.tile([128, 6], F32, tag="stats")
            nc.vector.bn_stats(out=stats, in_=ao_ps)
            mv = singles.tile([128, 2], F32, tag="mv")
            nc.vector.bn_aggr(out=mv, in_=stats)
            rstd = singles.tile([128, 1], F32, tag="rstd")
            nc.scalar.activation(
                rstd, mv[:, 1:2], mybir.ActivationFunctionType.Sqrt,
                bias=eps_sb, scale=1.0,
            )
            nc.vector.reciprocal(out=rstd, in_=rstd)

            tbl = singles.tile([128, 128], F32, tag="tbl")
            nc.vector.tensor_scalar(
                out=tbl, in0=ao_ps,
                scalar1=mv[:, 0:1], scalar2=rstd,
                op0=mybir.AluOpType.subtract, op1=mybir.AluOpType.mult,
            )
            tbl2 = singles.tile([128, 128], F32, tag="tbl2")
            nc.vector.tensor_mul(out=tbl2, in0=tbl, in1=gamma_gr)
            tbl3 = singles.tile([128, 128], F32, tag="tbl3")
            nc.vector.tensor_add(out=tbl3, in0=tbl2, in1=beta_gr)

            # table in matmul dtype; for "hilo", split into bf16 hi + lo
            tbl_hi = singles.tile([128, 128], OH_DT, tag="tbl_hi")
            nc.vector.tensor_copy(out=tbl_hi, in_=tbl3)
            tbl_lo = None
            if PRECISION == "hilo":
                hi32 = singles.tile([128, 128], F32, tag="hi32")
                nc.vector.tensor_copy(out=hi32, in_=tbl_hi)
                resid = singles.tile([128, 128], F32, tag="resid")
                nc.vector.tensor_sub(out=resid, in0=tbl3, in1=hi32)
                tbl_lo = singles.tile([128, 128], BF16, tag="tbl_lo")
                nc.vector.tensor_copy(out=tbl_lo, in_=resid)

            # ---------- main gather loop ----------
            nstores = (ngroups + STORE_G - 1) // STORE_G
            for s in range(nstores):
                gs = min(STORE_G, ngroups - s * STORE_G)
                stage = stage_pool.tile([128, STORE_G * GROUP], F32, tag="stage")
                use_dma_bc = BCAST == "hybrid" and s % 2 == 0
                if use_dma_bc:
                    # replicate uint8 idx across partitions via stride-0 DMA
                    # (reads the same 4KB DRAM row 128x; offloads the PE)
                    idx8_bc = stage_pool.tile(
                        [128, STORE_G * GROUP], mybir.dt.uint8, tag="idx8bc"
                    )
                    off = s * STORE_G * GROUP
                    src = bass.AP(
                        tensor=idx8_d[:].tensor, offset=off,
                        ap=[[0, 128], [1, gs * GROUP]],
                    )
                    nc.sync.dma_start(out=idx8_bc[:, :gs * GROUP], in_=src)
                for gi in range(gs):
                    g = s * STORE_G + gi
                    if use_dma_bc:
                        eq_in = idx8_bc[:, gi * GROUP:(gi + 1) * GROUP]
                    else:
                        # broadcast idx to all partitions via K=1 PE matmul
                        bc_ps = loop_ps.tile([128, GROUP], F32, tag="bcast")
                        nc.tensor.matmul(
                            bc_ps, onesbf,
                            idx_sb[:, g * GROUP:(g + 1) * GROUP],
                            start=True, stop=True,
                        )
                        eq_in = bc_ps
                    # onehotT[j, p] = (idx[p] == j)
                    oh = oh_pool.tile([128, GROUP], OH_DT, tag="oh")
                    nc.vector.tensor_scalar(
                        out=oh, in0=eq_in,
                        scalar1=iota_f, scalar2=None,
                        op0=mybir.AluOpType.is_equal,
                    )
                    # out[p, h] = sum_j onehotT[j, p] * table[j, h]
                    out_ps = loop_ps.tile([128, GROUP], F32, tag="outps")
                    for t in range(GROUP // 128):
                        sl = slice(t * 128, (t + 1) * 128)
                        if PRECISION == "hilo":
                            nc.tensor.matmul(
                                out_ps[:, sl], oh[:, sl], tbl_hi,
                                start=True, stop=False,
                            )
                            nc.tensor.matmul(
                                out_ps[:, sl], oh[:, sl], tbl_lo,
                                start=False, stop=True,
                            )
                        else:
                            nc.tensor.matmul(
                                out_ps[:, sl], oh[:, sl], tbl_hi,
                                start=True, stop=True,
                            )
                    nc.scalar.copy(
                        out=stage[:, gi * GROUP:(gi + 1) * GROUP], in_=out_ps
                    )
                # Node order is host-permuted so partition p owns DRAM rows
                # [p*NT, (p+1)*NT): every store descriptor is a contiguous
                # ts*512B run per partition (full DMA line rate).
                rows = gs * GROUP
                ts = rows // 128                      # tiles in this store
                t0 = s * (STORE_G * GROUP // 128)     # first tile of store
                dview = out_d[:, :].rearrange("(p t) c -> p t c", p=128)[
                    :, t0:t0 + ts, :
                ]
                sview = stage[:, :rows].rearrange("p (t c) -> p t c", c=128)
                nc.sync.dma_start(out=dview, in_=sview)

    nc.finalize()
    return nc


_CACHE = {}


def _get_nc(precision=None):
    key = precision or PRECISION
    if key not in _CACHE:
        _CACHE[key] = build_bass(precision=key)
    return _CACHE[key]


def _prep_in_maps(inputs):
    f32c = lambda x: np.ascontiguousarray(np.asarray(x), dtype=np.float32)
    win = f32c(inputs["Win"])
    bin_ = f32c(inputs["bin"])
    shared = {
        "graph_vec": f32c(inputs["graph_vec"]),
        "Wv": f32c(inputs["Wv"]),
        "bv": f32c(inputs["bv"]),
        "Wiv": f32c(win[2 * H:3 * H, :]),
        "biv": f32c(bin_[2 * H:3 * H]),
        "Wout": f32c(inputs["Wout"]),
        "bout": f32c(inputs["bout"]),
        "gamma": f32c(inputs["gamma"]),
        "beta": f32c(inputs["beta"]),
        "eye": np.eye(128, dtype=np.float32),
    }
    bi = np.asarray(inputs["batch_indices"]).astype(np.int64).reshape(N_CORES, NSHARD)
    idx_pad = np.zeros((N_CORES, NPAD), dtype=np.int64)
    idx_pad[:, :NSHARD] = bi
    # Permute so device tile t covers nodes {p*NT + t}: partition p then owns
    # the contiguous output-row block [p*NT, (p+1)*NT) (contiguous DMA runs).
    nt = NPAD // 128
    idx_tr = idx_pad.reshape(N_CORES, 128, nt).transpose(0, 2, 1)  # [c, t, p]
    idx_flat = idx_tr.reshape(N_CORES, NPAD)
    idx_bf = idx_flat.astype(ml_dtypes.bfloat16)  # exact: values < 256
    idx_u8 = idx_flat.astype(np.uint8)
    return [
        {
            **shared,
            "idx": np.ascontiguousarray(idx_bf[c]),
            "idx8": np.ascontiguousarray(idx_u8[c]),
        }
        for c in range(N_CORES)
    ]


def run_sharded(inputs, trace=False, precision=None, **kwargs):
    """Run the SPMD bass kernel on 8 cores; returns (output, BassKernelResults)."""
    in_maps = _prep_in_maps(inputs)
    nc = _get_nc(precision)
    res = bass_utils.run_bass_kernel_spmd(
        nc, in_maps, core_ids=list(range(N_CORES)), trace=trace, **kwargs
    )
    shards = [r["out"][:NSHARD] for r in res.results]
    out = np.concatenate(shards, axis=0)
    return out, res


def kernel(**inputs) -> np.ndarray:
    out, _ = run_sharded(inputs)
    return out

